# revision 34
# baseline (speedup 1.0000x reference)
"""Conv1d (B=32, C_in=C_out=64, L=16384, K=3, VALID) on 8 trn2 cores.

Strategy: data-parallel over batch (4 batches/core), polyphase-2 over L.
The host de-interleaves each batch's length axis into even/odd phases
stacked across 128 SBUF partitions: rows 0-63 = x[c, 0::2], rows
64-127 = x[c, 1::2].  The K=3 conv then needs only TWO PSUM-accumulated
matmuls per output chunk (vs 3 for the tap-per-matmul scheme):

  out_even(m) = w0 Xe[m] + w1 Xo[m] + w2 Xe[m+1]
  out_odd(m)  = w0 Xo[m] + w1 Xe[m+1] + w2 Xo[m+1]

  pass A: rhs = [Xe;Xo][:, m],   lhsT_A = [[w0^T, 0   ], [w1^T, w0^T]]
  pass B: rhs = [Xe;Xo][:, m+1], lhsT_B = [[w2^T, w1^T], [0,    w2^T]]

PSUM [128, n] = [out_even ch; out_odd ch]; the host re-interleaves.
This cuts TensorE busy ~48us -> ~30us, off the critical path.

DMA facts this kernel is shaped around (measured on this part):
 - The 16 SDMA engines serve the three DGE rings (sync-HWDGE,
   scalar-HWDGE, gpsimd-SWDGE) round-robin at DMA granularity;
   aggregate ~414 GB/s, ~158ns per 4KB packet, but packets under 4KB
   serialize at ~300ns each.  So all input/output rows are 4KB
   multiples (the output DRAM tensor is padded 8191 -> 8192 cols).
 - Each DMA's completion semaphore fires ~2us after its last byte.
 - A DMA trigger occupies its issuing engine's queue slot in strict
   FIFO order, so EVERY input trigger is issued up front (IBUFS=4
   holds all four batch tiles) - never behind an evacuation op or an
   output trigger.  Ring FIFO order then gives input strict priority
   on both HWDGE rings.
 - Outputs: batch 0 drains on the otherwise-idle SWDGE ring while
   input owns the HWDGE rings; every later chunk alternates
   scalar/sync, queued behind the remaining input transfers; the last
   batch tapers in 2048-col chunks so the final chunks drain in
   parallel.
 - The w+bias const rides SWDGE (it packs sub-4KB rows into 4KB
   packets, unlike HWDGE) during the HAM warm-up window.
PSUM->SBUF evacuation (fused bias add, fp32->fp16) splits each
512-col chunk across ACT and DVE so a PSUM bank recycles at matmul
pace and the PE never idles into a HAM re-throttle.
I/O is fp16 (~3e-4 rel err).  Shapes hardcoded from the spec.
"""

import os

import numpy as np

from concourse import bacc, bass, mybir, tile
from concourse.bass_utils import run_bass_kernel_spmd

B, C, L, K = 32, 64, 16384, 3
LOUT = L - K + 1  # 16382
NCORES = 8
BPC = B // NCORES  # 4 batches per core
P = 128  # partitions (2 phases x C)
LH = L // 2  # 8192 phase-cols per batch
MOUT = LOUT // 2  # 8191 output phase-cols per batch
MPAD = LH  # output cols padded to 8192 so all DMA rows are 4KB-clean
NJ = 512  # PSUM inner chunk (one fp32 bank)

F32 = mybir.dt.float32
F16 = mybir.dt.float16

IBUFS = int(os.environ.get("CONV_IBUFS", "4"))
OBUFS = int(os.environ.get("CONV_OBUFS", "12"))
WARMUP = int(os.environ.get("CONV_WARMUP", "7"))

# input sub-DMA col counts per batch: 2048-col subs (4KB rows) keep
# the PE's burst-idle gaps under the ~3.4us HAM re-throttle window
# (a 4096-col sub gives 3.4us compute bursts separated by ~5us of
# DMA wait -> the PE clock-gate oscillates and matmuls run at half
# clock; measured throttle_active ~18us with 4096-col subs)
IN_SUBS = {}
for _p in range(BPC):
    IN_SUBS[_p] = [2048, 2048, 2048, 2048]
# output chunk col counts per batch (sum to MPAD)
OUT_CHUNKS = {}
for _p in range(BPC):
    OUT_CHUNKS[_p] = [2048, 2048, 2048, 2048]
# output ring per global output-chunk index: ALL outputs queue behind
# the input transfers on the two HWDGE rings (ring FIFO = strict input
# wire priority, so the 8.4MB input lands by ~29us and the PE runs on
# a continuous backlog - staying HAM-warm - instead of input-gated
# bursts that let the clock gate oscillate).  The output backlog sits
# in SBUF (OBUFS=12 tiles) and drains at full fabric rate once input
# is done.  Only the last two chunks ride the by-then-idle SWDGE ring
# so the final drain uses all three rings in parallel.
OUT_ENG = ["scalar", "sync", "scalar", "sync",
           "scalar", "sync", "scalar", "sync",
           "scalar", "sync", "scalar", "sync",
           "scalar", "sync", "gpsimd", "gpsimd"]

_NC_CACHE = []


def _build_nc():
    nc = bacc.Bacc("TRN2", target_bir_lowering=False, debug=False,
                   num_devices=NCORES)

    x2 = nc.dram_tensor("x2", [BPC, P, LH], F16, kind="ExternalInput")
    wb = nc.dram_tensor("wb", [P, 2 * P + 4], F16, kind="ExternalInput")
    y2 = nc.dram_tensor("y2", [BPC, P, MPAD], F16, kind="ExternalOutput")

    with tile.TileContext(nc) as tc:
        with (
            tc.tile_pool(name="const", bufs=1) as const_pool,
            tc.tile_pool(name="inp", bufs=IBUFS) as inp_pool,
            tc.tile_pool(name="outp", bufs=OBUFS) as outp_pool,
            tc.tile_pool(name="psum", bufs=8, space=bass.MemorySpace.PSUM)
            as psum_pool,
        ):
            # consts on SWDGE (idle until batch 0's outputs); it packs
            # the 520B rows into 4KB packets, and the data arrives
            # during the HAM warm-up window.
            wbt = const_pool.tile([P, 2 * P + 4], F16)
            nc.gpsimd.dma_start(out=wbt[:], in_=wb[:])
            wA = wbt[:, 0:P]
            wB = wbt[:, P:2 * P]
            # fp32 bias stored in two f16 slots (DVE tensor_scalar
            # needs an fp32 scalar operand)
            bias = wbt[:, 2 * P:2 * P + 2].bitcast(F32)

            # ALL input sub-DMAs issued up front, alternating the two
            # HWDGE rings.
            tiles = {}
            isel = 0
            for p in range(BPC):
                it = inp_pool.tile([P, LH], F16, tag="in")
                c0 = 0
                for n in IN_SUBS[p]:
                    eng = nc.sync if isel % 2 == 0 else nc.scalar
                    eng.dma_start(out=it[:, c0:c0 + n],
                                  in_=x2[p, :, c0:c0 + n])
                    isel += 1
                    c0 += n
                tiles[p] = it

            # HAM warm-up: dummy matmuls on zeroed SBUF while the first
            # input DMA is in flight, so the PE clock gate is at 8/8
            # (2.4 GHz) when real work arrives.
            if WARMUP:
                wz = const_pool.tile([P, NJ], F16)
                nc.vector.memset(wz[:], 0.0)
                for i in range(WARMUP):
                    wp = psum_pool.tile([P, NJ], F32, tag="acc",
                                        name=f"warm{i}")
                    nc.tensor.matmul(wp[:], wz[:, :P], wz[:],
                                     start=True, stop=True)

            oi = 0  # global output-chunk index (ring assignment)
            for p in range(BPC):
                it = tiles.pop(p)
                m0 = 0
                for n in OUT_CHUNKS[p]:
                    ot = outp_pool.tile([P, 4096], F16, tag="out")
                    for j0 in range(m0, min(m0 + n, MOUT), NJ):
                        nj = min(NJ, MOUT - j0)
                        o0 = j0 - m0
                        pt = psum_pool.tile([P, NJ], F32, tag="acc")
                        nc.tensor.matmul(pt[:, :nj], wA,
                                         it[:, j0:j0 + nj],
                                         start=True, stop=False)
                        nc.tensor.matmul(pt[:, :nj], wB,
                                         it[:, j0 + 1:j0 + 1 + nj],
                                         start=False, stop=True)
                        # psum -> sbuf with fused bias add, split
                        # ACT/DVE so the bank frees at matmul pace
                        h = nj // 2
                        nc.scalar.add(ot[:, o0:o0 + h], pt[:, :h],
                                      add=bias)
                        nc.vector.tensor_scalar_add(ot[:, o0 + h:o0 + nj],
                                                    pt[:, h:nj],
                                                    bias)
                    if m0 + n == MPAD:
                        # pad col (host ignores) so the DMA reads only
                        # written SBUF and rows stay 4KB-multiples
                        nc.vector.memset(ot[:, n - 1:n], 0.0)
                    eng = {"sync": nc.sync, "scalar": nc.scalar,
                           "gpsimd": nc.gpsimd}[OUT_ENG[oi]]
                    eng.dma_start(out=y2[p, :, m0:m0 + n], in_=ot[:, :n])
                    oi += 1
                    m0 += n

    nc.compile()
    return nc


def _get_nc():
    if not _NC_CACHE:
        _NC_CACHE.append(_build_nc())
    return _NC_CACHE[0]


def _prep_weights(weight, bias):
    w = weight.astype(np.float32)
    wb = np.zeros((P, 2 * P + 4), np.float32)
    w0, w1, w2 = w[:, :, 0].T, w[:, :, 1].T, w[:, :, 2].T  # [C_in, C_out]
    wb[0:C, 0:C] = w0
    wb[C:P, 0:C] = w1
    wb[C:P, C:P] = w0
    wb[0:C, P:P + C] = w2
    wb[0:C, P + C:2 * P] = w1
    wb[C:P, P + C:2 * P] = w2
    wb16 = wb.astype(np.float16)
    # fp32 bias bit-packed into f16 slots 256:258
    wb16[:, 2 * P:2 * P + 2].view(np.float32)[:, 0] = np.concatenate(
        [bias, bias]).astype(np.float32)
    return wb16


def kernel(x, weight, bias, _want_results=False, **run_kwargs):
    x = np.asarray(x, np.float32)
    weight = np.asarray(weight, np.float32)
    bias = np.asarray(bias, np.float32)
    nc = _get_nc()
    wb = _prep_weights(weight, bias)

    # de-interleave length into even/odd phases stacked on partitions
    xh = x.astype(np.float16)
    in_maps = []
    for i in range(NCORES):
        xs = xh[BPC * i:BPC * (i + 1)]  # [BPC, C, L]
        xde = np.empty((BPC, P, LH), np.float16)
        xde[:, :C, :] = xs[:, :, 0::2]
        xde[:, C:, :] = xs[:, :, 1::2]
        in_maps.append({"x2": xde, "wb": wb})

    res = run_bass_kernel_spmd(nc, in_maps, list(range(NCORES)), **run_kwargs)

    out = np.empty((B, C, LOUT), np.float32)
    for i in range(NCORES):
        yde = res.results[i]["y2"][:, :, :MOUT]  # drop pad col
        ob = out[BPC * i:BPC * (i + 1)]
        ob[:, :, 0::2] = yde[:, :C, :]
        ob[:, :, 1::2] = yde[:, C:, :]
    if _want_results:
        return out, res
    return out


# revision 35
# speedup vs baseline: 1.0907x; 1.0907x over previous
"""Conv1d (B=32, C_in=C_out=64, L=16384, K=3, VALID) on 8 trn2 cores.

Strategy: data-parallel over batch (4 batches/core), polyphase-2 over L.
The host de-interleaves each batch's length axis into even/odd phases
stacked across 128 SBUF partitions: rows 0-63 = x[c, 0::2], rows
64-127 = x[c, 1::2].  The K=3 conv then needs only TWO PSUM-accumulated
matmuls per output chunk (vs 3 for the tap-per-matmul scheme):

  out_even(m) = w0 Xe[m] + w1 Xo[m] + w2 Xe[m+1]
  out_odd(m)  = w0 Xo[m] + w1 Xe[m+1] + w2 Xo[m+1]

  pass A: rhs = [Xe;Xo][:, m],   lhsT_A = [[w0^T, 0   ], [w1^T, w0^T]]
  pass B: rhs = [Xe;Xo][:, m+1], lhsT_B = [[w2^T, w1^T], [0,    w2^T]]

PSUM [128, n] = [out_even ch; out_odd ch]; the host re-interleaves.
This cuts TensorE busy ~48us -> ~30us, off the critical path.

DMA facts this kernel is shaped around (measured on this part):
 - The 16 SDMA engines serve the three DGE rings (sync-HWDGE,
   scalar-HWDGE, gpsimd-SWDGE) round-robin at DMA granularity;
   aggregate ~414 GB/s, ~158ns per 4KB packet, but packets under 4KB
   serialize at ~300ns each, and the SWDGE ring only sustains
   ~165 GB/s.  So all input/output rows are 4KB multiples (the output
   DRAM tensor is padded 8191 -> 8192 cols).
 - Each DMA's completion semaphore fires ~2us after its last byte.
 - A DMA trigger occupies its issuing engine's queue slot in strict
   FIFO order, so EVERY input trigger is issued up front (IBUFS=4
   holds all four batch tiles) - never behind an evacuation op or an
   output trigger.  Ring FIFO order then gives input strict priority
   on both HWDGE rings.
 - Outputs: batch 0 drains on the otherwise-idle SWDGE ring while
   input owns the HWDGE rings; every later chunk alternates
   scalar/sync, queued behind the remaining input transfers; the last
   batch tapers in 2048-col chunks so the final chunks drain in
   parallel.
 - The w+bias const rides SWDGE (it packs sub-4KB rows into 4KB
   packets, unlike HWDGE) during the HAM warm-up window.
PSUM->SBUF evacuation (fused bias add, fp32->fp16) splits each
512-col chunk across ACT and DVE so a PSUM bank recycles at matmul
pace and the PE never idles into a HAM re-throttle.
I/O is fp16 (~3e-4 rel err).  Shapes hardcoded from the spec.
"""

import os

import numpy as np

from concourse import bacc, bass, mybir, tile
from concourse.bass_utils import run_bass_kernel_spmd

B, C, L, K = 32, 64, 16384, 3
LOUT = L - K + 1  # 16382
NCORES = 8
BPC = B // NCORES  # 4 batches per core
P = 128  # partitions (2 phases x C)
LH = L // 2  # 8192 phase-cols per batch
MOUT = LOUT // 2  # 8191 output phase-cols per batch
MPAD = LH  # output cols padded to 8192 so all DMA rows are 4KB-clean
NJ = 512  # PSUM inner chunk (one fp32 bank)

F32 = mybir.dt.float32
F16 = mybir.dt.float16

IBUFS = int(os.environ.get("CONV_IBUFS", "4"))
OBUFS = int(os.environ.get("CONV_OBUFS", "8"))
WARMUP = int(os.environ.get("CONV_WARMUP", "7"))

# input sub-DMA col counts per batch: 2048-col subs (4KB rows) keep
# the PE's burst-idle gaps under the ~3.4us HAM re-throttle window
# (a 4096-col sub gives 3.4us compute bursts separated by ~5us of
# DMA wait -> the PE clock-gate oscillates and matmuls run at half
# clock; measured throttle_active ~18us with 4096-col subs)
IN_SUBS = {}
for _p in range(BPC):
    IN_SUBS[_p] = [2048, 2048, 2048, 2048]
# output chunk col counts per batch (sum to MPAD)
OUT_CHUNKS = {}
for _p in range(BPC):
    OUT_CHUNKS[_p] = [2048, 2048, 2048, 2048]
# output ring per global output-chunk index: batch 0 on the slow,
# otherwise-idle SWDGE ring; later chunks alternate the HWDGE rings
# where they queue behind the remaining input transfers (FIFO =
# input priority; all input triggers are issued pre-loop)
OUT_ENG = ["gpsimd", "gpsimd", "gpsimd", "gpsimd",
           "scalar", "sync", "scalar", "sync",
           "scalar", "sync", "scalar", "sync",
           "scalar", "sync", "scalar", "sync"]

_NC_CACHE = []


def _build_nc():
    nc = bacc.Bacc("TRN2", target_bir_lowering=False, debug=False,
                   num_devices=NCORES)

    x2 = nc.dram_tensor("x2", [BPC, P, LH], F16, kind="ExternalInput")
    wb = nc.dram_tensor("wb", [P, 2 * P + 4], F16, kind="ExternalInput")
    y2 = nc.dram_tensor("y2", [BPC, P, MPAD], F16, kind="ExternalOutput")

    with tile.TileContext(nc) as tc:
        with (
            tc.tile_pool(name="const", bufs=1) as const_pool,
            tc.tile_pool(name="inp", bufs=IBUFS) as inp_pool,
            tc.tile_pool(name="outp", bufs=OBUFS) as outp_pool,
            tc.tile_pool(name="psum", bufs=8, space=bass.MemorySpace.PSUM)
            as psum_pool,
        ):
            # consts on SWDGE (idle until batch 0's outputs); it packs
            # the 520B rows into 4KB packets, and the data arrives
            # during the HAM warm-up window.
            wbt = const_pool.tile([P, 2 * P + 4], F16)
            nc.gpsimd.dma_start(out=wbt[:], in_=wb[:])
            wA = wbt[:, 0:P]
            wB = wbt[:, P:2 * P]
            # fp32 bias stored in two f16 slots (DVE tensor_scalar
            # needs an fp32 scalar operand)
            bias = wbt[:, 2 * P:2 * P + 2].bitcast(F32)

            # ALL input sub-DMAs issued up front, alternating the two
            # HWDGE rings.
            tiles = {}
            isel = 0
            for p in range(BPC):
                it = inp_pool.tile([P, LH], F16, tag="in")
                c0 = 0
                for n in IN_SUBS[p]:
                    eng = nc.sync if isel % 2 == 0 else nc.scalar
                    eng.dma_start(out=it[:, c0:c0 + n],
                                  in_=x2[p, :, c0:c0 + n])
                    isel += 1
                    c0 += n
                tiles[p] = it

            # HAM warm-up: dummy matmuls on zeroed SBUF while the first
            # input DMA is in flight, so the PE clock gate is at 8/8
            # (2.4 GHz) when real work arrives.
            if WARMUP:
                wz = const_pool.tile([P, NJ], F16)
                nc.vector.memset(wz[:], 0.0)
                for i in range(WARMUP):
                    wp = psum_pool.tile([P, NJ], F32, tag="acc",
                                        name=f"warm{i}")
                    nc.tensor.matmul(wp[:], wz[:, :P], wz[:],
                                     start=True, stop=True)

            oi = 0  # global output-chunk index (ring assignment)
            for p in range(BPC):
                it = tiles.pop(p)
                m0 = 0
                for n in OUT_CHUNKS[p]:
                    ot = outp_pool.tile([P, 4096], F16, tag="out")
                    for j0 in range(m0, min(m0 + n, MOUT), NJ):
                        nj = min(NJ, MOUT - j0)
                        o0 = j0 - m0
                        pt = psum_pool.tile([P, NJ], F32, tag="acc")
                        nc.tensor.matmul(pt[:, :nj], wA,
                                         it[:, j0:j0 + nj],
                                         start=True, stop=False)
                        nc.tensor.matmul(pt[:, :nj], wB,
                                         it[:, j0 + 1:j0 + 1 + nj],
                                         start=False, stop=True)
                        # psum -> sbuf with fused bias add, split
                        # ACT/DVE so the bank frees at matmul pace
                        h = nj // 2
                        nc.scalar.add(ot[:, o0:o0 + h], pt[:, :h],
                                      add=bias)
                        nc.vector.tensor_scalar_add(ot[:, o0 + h:o0 + nj],
                                                    pt[:, h:nj],
                                                    bias)
                    if m0 + n == MPAD:
                        # pad col (host ignores) so the DMA reads only
                        # written SBUF and rows stay 4KB-multiples
                        nc.vector.memset(ot[:, n - 1:n], 0.0)
                    eng = {"sync": nc.sync, "scalar": nc.scalar,
                           "gpsimd": nc.gpsimd}[OUT_ENG[oi]]
                    eng.dma_start(out=y2[p, :, m0:m0 + n], in_=ot[:, :n])
                    oi += 1
                    m0 += n

    nc.compile()
    return nc


def _get_nc():
    if not _NC_CACHE:
        _NC_CACHE.append(_build_nc())
    return _NC_CACHE[0]


def _prep_weights(weight, bias):
    w = weight.astype(np.float32)
    wb = np.zeros((P, 2 * P + 4), np.float32)
    w0, w1, w2 = w[:, :, 0].T, w[:, :, 1].T, w[:, :, 2].T  # [C_in, C_out]
    wb[0:C, 0:C] = w0
    wb[C:P, 0:C] = w1
    wb[C:P, C:P] = w0
    wb[0:C, P:P + C] = w2
    wb[0:C, P + C:2 * P] = w1
    wb[C:P, P + C:2 * P] = w2
    wb16 = wb.astype(np.float16)
    # fp32 bias bit-packed into f16 slots 256:258
    wb16[:, 2 * P:2 * P + 2].view(np.float32)[:, 0] = np.concatenate(
        [bias, bias]).astype(np.float32)
    return wb16


def kernel(x, weight, bias, _want_results=False, **run_kwargs):
    x = np.asarray(x, np.float32)
    weight = np.asarray(weight, np.float32)
    bias = np.asarray(bias, np.float32)
    nc = _get_nc()
    wb = _prep_weights(weight, bias)

    # de-interleave length into even/odd phases stacked on partitions
    xh = x.astype(np.float16)
    in_maps = []
    for i in range(NCORES):
        xs = xh[BPC * i:BPC * (i + 1)]  # [BPC, C, L]
        xde = np.empty((BPC, P, LH), np.float16)
        xde[:, :C, :] = xs[:, :, 0::2]
        xde[:, C:, :] = xs[:, :, 1::2]
        in_maps.append({"x2": xde, "wb": wb})

    res = run_bass_kernel_spmd(nc, in_maps, list(range(NCORES)), **run_kwargs)

    out = np.empty((B, C, LOUT), np.float32)
    for i in range(NCORES):
        yde = res.results[i]["y2"][:, :, :MOUT]  # drop pad col
        ob = out[BPC * i:BPC * (i + 1)]
        ob[:, :, 0::2] = yde[:, :C, :]
        ob[:, :, 1::2] = yde[:, C:, :]
    if _want_results:
        return out, res
    return out
